# revision 4
# baseline (speedup 1.0000x reference)
"""ChebConv-with-spatial-attention Trainium2 kernel (8 NeuronCores, SPMD data-parallel).

Math (per batch b):
    M_k = cheb[k] * att[b]              (elementwise, [N,N])
    R_k = M_k @ xmat[b]                 (xmat[b][j, t*F+f] = x[b,t,j,f], [N, T*F])
    out[b,t,i,o] = relu( sum_k sum_f R_k[i, t*F+f] * Theta[k,f,o] )

Device mapping (per core, 2 batches), fp8 DoubleRow scheme:
    stage 1: host ships Mt = (cheb_k*att_b)^T pre-scaled, quantized as an
             fp8e4m3 (hi, lo) pair with lo = fp8(M - hi), so hi+lo carries
             ~bf16 accuracy.  PE DoubleRow matmuls contract both slots at
             0.5 cycles/column:
                 R_T[tf, i] += xh8[j, tf].T (slot c) @ Mpair[j, c, i]
             with xh8 = fp8(x) duplicated across the two slots.  This halves
             stage-1 PE time vs bf16 while keeping max-rel error ~1.4e-2
             (error dominated by the single fp8 rounding of x).
    stage 2: bf16 PE matmuls out[i, (t,o)] += R_T[tf_blk, i].T @ thetap[k]
             with thetap a block-diagonal padded Theta ([128, 4*64] per k,
             scale 1/s_k folded in), accumulated over k in PSUM; fused ReLU
             on copy-out; bf16 output stores (host casts back to f32).

Host pre-processing: all inputs packed as SBUF images [128, cols] so every
DMA moves >=512B contiguous rows at the full modeled bandwidth.  T_0 of any
Chebyshev basis is the identity, so its term reduces to a diagonal
(attention-diag) scaling folded into `xht` (bf16) on the host.
"""

import numpy as np

B, T, N, F_IN, F_OUT, K = 16, 12, 1024, 32, 64, 3
M_CORES = 8
NB = B // M_CORES          # batches per core
P = 128                    # SBUF partitions
NJ = N // P                # 8 contraction chunks
TF = T * F_IN              # 384
NTFB = TF // P             # 3 tf blocks
TBLK = P // F_IN           # 4 t's per tf block
IS = 512                   # stage-1 moving width
NIS = N // IS              # 2 i strips
TO = TBLK * F_OUT          # 256 = stage-2 rhs width

_cache = {}


def _build(fast_k0=True, reps=1):
    import concourse.bacc as bacc
    import concourse.mybir as mybir
    import concourse.tile as tile

    F8 = mybir.dt.float8e4
    BF = mybir.dt.bfloat16
    F32 = mybir.dt.float32
    DR = mybir.MatmulPerfMode.DoubleRow
    KM = K - 1 if fast_k0 else K   # k's that need stage-1 matmuls

    nc = bacc.Bacc("TRN2", target_bir_lowering=False, debug=False)
    # SBUF-image layouts (see kernel() for the exact packing)
    m8_d = nc.dram_tensor("m8", [NB, KM, P, NJ * NIS * 2 * IS], F8,
                          kind="ExternalInput")
    xh8_d = nc.dram_tensor("xh8", [NB, P, NJ * NTFB * 2 * P], F8,
                           kind="ExternalInput")
    thp_d = nc.dram_tensor("thp", [P, K * TO], BF, kind="ExternalInput")
    if fast_k0:
        xht_d = nc.dram_tensor("xht", [NB, P, NTFB * N], BF,
                               kind="ExternalInput")
    # device layout [b, i, (t, o)]; host permutes back to [b, t, i, o]
    out_d = nc.dram_tensor("out", [NB, N, T * F_OUT], BF, kind="ExternalOutput")

    with tile.TileContext(nc) as tc:
        with (
            tc.tile_pool(name="m8", bufs=3) as m8_pool,
            tc.tile_pool(name="xh8", bufs=2) as xh8_pool,
            tc.tile_pool(name="xht", bufs=2) as xht_pool,
            tc.tile_pool(name="rt", bufs=2) as rt_pool,
            tc.tile_pool(name="thp", bufs=1) as thp_pool,
            tc.tile_pool(name="osb", bufs=3) as out_pool,
            tc.tile_pool(name="rtps", bufs=2, space="PSUM") as rtps_pool,
            tc.tile_pool(name="outps", bufs=2, space="PSUM") as outps_pool,
        ):
            import itertools

            # Only SP and Pool issue DMAs: ACT/DVE FIFOs must stay clear for
            # the PSUM->SBUF copies and ReLUs.
            _rr = itertools.count()
            _engs = [nc.sync, nc.gpsimd]

            def dma(dst, src):
                _engs[next(_rr) % 2].dma_start(dst, src)

            thp_sb = thp_pool.tile([P, K * TO], BF, tag="thp")

            m8_tiles = {}
            xh8_tiles = {}
            xht_tiles = {}

            def load_xh8(b):
                sb = xh8_pool.tile([P, NJ * NTFB * 2 * P], F8, tag="xh8",
                                   name="xh8_sb")
                w = NTFB * 2 * P
                for jb in range(NJ):
                    dma(sb[:, jb * w:(jb + 1) * w],
                        xh8_d.ap()[b][:, jb * w:(jb + 1) * w])
                xh8_tiles[b] = sb

            def load_m8(b, km):
                sb = m8_pool.tile([P, NJ * NIS * 2 * IS], F8, tag=f"m8_{km}",
                                  name=f"m8_{km}")
                w = NIS * 2 * IS
                for jb in range(NJ):
                    dma(sb[:, jb * w:(jb + 1) * w],
                        m8_d.ap()[b][km][:, jb * w:(jb + 1) * w])
                m8_tiles[(b, km)] = sb

            def load_xht(b):
                sb = xht_pool.tile([P, NTFB * N], BF, tag="xht", name="xht_sb")
                for tfb in range(NTFB):
                    dma(sb[:, tfb * N:(tfb + 1) * N],
                        xht_d.ap()[b][:, tfb * N:(tfb + 1) * N])
                xht_tiles[b] = sb

            for rep in range(reps):
                if rep == 0:
                    dma(thp_sb[:], thp_d.ap())
                # need-order loads: stage-1(b,km) needs xh8(b)+m8(b,km);
                # stage-2(b) needs xht(b); then the next b.
                for b in range(NB):
                    load_xh8(b)
                    for km in range(KM):
                        load_m8(b, km)
                    if fast_k0:
                        load_xht(b)

                for b in range(NB):
                    xh8_sb = xh8_tiles[b]

                    # stage 1: R_T via fp8 DoubleRow, bf16 out in SBUF
                    rt_sb = rt_pool.tile([P, KM * NTFB * N], BF, tag="rt")
                    cp = itertools.count()
                    for km in range(KM):
                        m8_sb = m8_tiles[(b, km)]
                        for tfb in range(NTFB):
                            rtps = [
                                rtps_pool.tile([P, IS], F32, tag=f"rtps{q}",
                                               name=f"rtps{q}")
                                for q in range(NIS)
                            ]
                            for jb in range(NJ):
                                base_l = (jb * NTFB + tfb) * 2 * P
                                lhsT = xh8_sb[:, base_l:base_l + 2 * P].rearrange(
                                    "p (c m) -> p c m", c=2)
                                for q in range(NIS):
                                    base_r = (jb * NIS + q) * 2 * IS
                                    rhs = m8_sb[:, base_r:base_r + 2 * IS].rearrange(
                                        "p (c n) -> p c n", c=2)
                                    nc.tensor.matmul(
                                        rtps[q][:], lhsT, rhs,
                                        start=(jb == 0), stop=(jb == NJ - 1),
                                        perf_mode=DR,
                                    )
                            for q in range(NIS):
                                dst = rt_sb[:, (km * NTFB + tfb) * N + q * IS:
                                            (km * NTFB + tfb) * N + (q + 1) * IS]
                                if next(cp) % 2 == 0:
                                    nc.scalar.copy(dst, rtps[q][:])
                                else:
                                    nc.vector.tensor_copy(dst, rtps[q][:])

                    # stage 2: out[i, (t,o)] accumulated over k per tf block
                    if fast_k0:
                        xht_sb = xht_tiles[b]
                    for ic in range(NJ):
                        ops_a = outps_pool.tile([P, 2 * TO], F32, tag="outpsA",
                                                name="ops_a")
                        ops_b = outps_pool.tile([P, TO], F32, tag="outpsB",
                                                name="ops_b")
                        for tfb in range(NTFB):
                            for k in range(K):
                                if fast_k0 and k == 0:
                                    lhs2 = xht_sb[:, tfb * N + ic * P:
                                                  tfb * N + ic * P + P]
                                else:
                                    km = k - 1 if fast_k0 else k
                                    base = (km * NTFB + tfb) * N + ic * P
                                    lhs2 = rt_sb[:, base:base + P]
                                dst = (ops_a[:, tfb * TO:(tfb + 1) * TO]
                                       if tfb < 2 else ops_b[:])
                                nc.tensor.matmul(
                                    dst, lhs2, thp_sb[:, k * TO:(k + 1) * TO],
                                    start=(k == 0), stop=(k == K - 1),
                                )
                        osb = out_pool.tile([P, T * F_OUT], BF, tag="osb")
                        if ic % 2 == 0:
                            nc.scalar.activation(
                                osb[:, :2 * TO], ops_a[:],
                                mybir.ActivationFunctionType.Relu)
                            nc.vector.tensor_relu(osb[:, 2 * TO:], ops_b[:])
                        else:
                            nc.vector.tensor_relu(osb[:, :2 * TO], ops_a[:])
                            nc.scalar.activation(
                                osb[:, 2 * TO:], ops_b[:],
                                mybir.ActivationFunctionType.Relu)
                        dma(out_d.ap()[b][ic * P:(ic + 1) * P, :], osb[:])

    nc.compile()
    return nc


def _pow2_scale(maxabs):
    """Largest power of 2 s with maxabs * s <= 224 (fp8e4m3 headroom)."""
    if maxabs <= 0:
        return 1.0
    return float(2.0 ** np.floor(np.log2(224.0 / maxabs)))


def kernel(x, spatial_attention, cheb, Theta):
    from ml_dtypes import bfloat16, float8_e4m3
    from concourse.bass_utils import run_bass_kernel_spmd

    x = np.asarray(x, dtype=np.float32)
    att = np.asarray(spatial_attention, dtype=np.float32)
    cheb = np.asarray(cheb, dtype=np.float32)
    Theta = np.asarray(Theta, dtype=np.float32)

    fast_k0 = bool(np.abs(cheb[0] - np.eye(N, dtype=np.float32)).max() <= 1e-6)
    kms = list(range(1, K)) if fast_k0 else list(range(K))
    KM = len(kms)

    key = "fast" if fast_k0 else "general"
    if key not in _cache:
        _cache[key] = _build(fast_k0=fast_k0)
    nc = _cache[key]

    # ---- xh8: fp8(x), [j, tf] chunks duplicated across DoubleRow slots ----
    xh = np.ascontiguousarray(x.transpose(0, 2, 1, 3).reshape(B, N, TF))
    xs = _pow2_scale(np.abs(xh).max()) / 32.0   # keep typical |x| ~ O(1)
    xs = min(xs, 1.0)
    v = (xh * xs).astype(float8_e4m3)
    v = v.reshape(B, NJ, P, NTFB, P).transpose(0, 2, 1, 3, 4)  # [b,p,jb,tfb,m]
    xh8 = np.stack([v, v], axis=4)  # [b,p,jb,tfb,c,m] -> dup slots
    xh8 = np.ascontiguousarray(
        xh8.transpose(0, 1, 2, 3, 4, 5).reshape(B, P, NJ * NTFB * 2 * P))

    # ---- m8: (cheb_k * att_b)^T scaled, fp8 (hi, lo) pair ----
    sk = {}
    m8 = np.empty((B, KM, P, NJ * NIS * 2 * IS), dtype=float8_e4m3)
    for idx, k in enumerate(kms):
        mt = (cheb[k][None] * att).transpose(0, 2, 1)  # [b, j, i]
        s = _pow2_scale(np.abs(mt).max())
        sk[k] = s
        mts = mt * s
        hi = mts.astype(float8_e4m3)
        lo = (mts - hi.astype(np.float32)).astype(float8_e4m3)
        # layout: [b, p, jb, q, c, n] with j = jb*P+p, i = q*IS+n
        pr = np.stack(
            [hi.reshape(B, NJ, P, NIS, IS), lo.reshape(B, NJ, P, NIS, IS)],
            axis=4)  # [b, jb, p, q, c, n]
        m8[:, idx] = pr.transpose(0, 2, 1, 3, 4, 5).reshape(B, P, -1)

    # ---- thetap: block-diag padded Theta with 1/s_k folded in ----
    thp = np.zeros((P, K * TO), dtype=np.float32)
    for kk in range(K):
        if fast_k0 and kk == 0:
            sc = 1.0
        else:
            sc = 1.0 / (sk[kk] * xs) if kk in sk else 1.0
        for tr in range(TBLK):
            thp[tr * F_IN:(tr + 1) * F_IN,
                kk * TO + tr * F_OUT:kk * TO + (tr + 1) * F_OUT] = Theta[kk] * sc
    thp = thp.astype(bfloat16)

    in_maps = [
        {
            "m8": m8[c * NB:(c + 1) * NB],
            "xh8": xh8[c * NB:(c + 1) * NB],
            "thp": thp,
        }
        for c in range(M_CORES)
    ]
    if fast_k0:
        # xht[b, tf, i] = x[b,t,i,f] * att[b,i,i], packed [b, p, tfb*N + i]
        attd = np.einsum("bii->bi", att)
        w = (x.transpose(0, 1, 3, 2).reshape(B, TF, N) * attd[:, None, :])
        xht = np.ascontiguousarray(
            w.reshape(B, NTFB, P, N).transpose(0, 2, 1, 3).reshape(B, P, NTFB * N)
        ).astype(bfloat16)
        for c in range(M_CORES):
            in_maps[c]["xht"] = xht[c * NB:(c + 1) * NB]

    try:
        res = run_bass_kernel_spmd(nc, in_maps, list(range(M_CORES)))
    except Exception:
        # transient NRT device hiccups recover on redispatch
        res = run_bass_kernel_spmd(nc, in_maps, list(range(M_CORES)))
    out = np.concatenate([res.results[c]["out"] for c in range(M_CORES)], axis=0)
    # device layout [b, i, (t, o)] -> [b, t, i, o], bf16 -> f32
    out = out.astype(np.float32).reshape(B, N, T, F_OUT).transpose(0, 2, 1, 3)
    return np.ascontiguousarray(out)


# revision 5
# speedup vs baseline: 1.0775x; 1.0775x over previous
"""ChebConv-with-spatial-attention Trainium2 kernel (8 NeuronCores, SPMD data-parallel).

Math (per batch b):
    M_k = cheb[k] * att[b]              (elementwise, [N,N])
    R_k = M_k @ xmat[b]                 (xmat[b][j, t*F+f] = x[b,t,j,f], [N, T*F])
    out[b,t,i,o] = relu( sum_k sum_f R_k[i, t*F+f] * Theta[k,f,o] )

Device mapping (per core, 2 batches), fp8 DoubleRow scheme:
    stage 1: host ships Mt = (cheb_k*att_b)^T pre-scaled, quantized as an
             fp8e4m3 (hi, lo) pair with lo = fp8(M - hi), so hi+lo carries
             ~bf16 accuracy.  PE DoubleRow matmuls contract both slots at
             0.5 cycles/column:
                 R_T[tf, i] += xh8[j, tf].T (bcast slots) @ Mpair[j, c, i]
             with xh8 = fp8(x) broadcast across the two slots (stride-0 AP).
             This halves stage-1 PE time vs bf16 while keeping max-rel error
             ~1.4e-2 (dominated by the single fp8 rounding of x).
    stage 2: bf16 PE matmuls out[i, (t,o)] += R_T[tf_blk, i].T @ thetap[k]
             with thetap a block-diagonal padded Theta ([128, 4*64] per k,
             scale 1/s_k folded in), accumulated over k in PSUM; fused ReLU
             on copy-out; bf16 output stores (host casts back to f32).

Host pre-processing: all inputs packed as SBUF images [128, cols] so every
DMA moves >=512B contiguous rows at the full modeled bandwidth; loads are
issued in need-order with (xh8, m8) chunk interleaving so the PE starts
within ~2us.  T_0 of any Chebyshev basis is the identity, so its term
reduces to a diagonal (attention-diag) scaling folded into `xht` (bf16).
"""

import numpy as np

B, T, N, F_IN, F_OUT, K = 16, 12, 1024, 32, 64, 3
M_CORES = 8
NB = B // M_CORES          # batches per core
P = 128                    # SBUF partitions
NJ = N // P                # 8 contraction chunks
TF = T * F_IN              # 384
NTFB = TF // P             # 3 tf blocks
TBLK = P // F_IN           # 4 t's per tf block
IS = 512                   # stage-1 moving width
NIS = N // IS              # 2 i strips
TO = TBLK * F_OUT          # 256 = stage-2 rhs width

_cache = {}


def _build(fast_k0=True, reps=1):
    import concourse.bacc as bacc
    import concourse.mybir as mybir
    import concourse.tile as tile

    F8 = mybir.dt.float8e4
    BF = mybir.dt.bfloat16
    F32 = mybir.dt.float32
    DR = mybir.MatmulPerfMode.DoubleRow
    KM = K - 1 if fast_k0 else K   # k's that need stage-1 matmuls

    nc = bacc.Bacc("TRN2", target_bir_lowering=False, debug=False)
    m8_d = nc.dram_tensor("m8", [NB, KM, P, NJ * NIS * 2 * IS], F8,
                          kind="ExternalInput")
    xh8_d = nc.dram_tensor("xh8", [NB, P, NJ * NTFB * P], F8,
                           kind="ExternalInput")
    thp_d = nc.dram_tensor("thp", [P, K * TO], BF, kind="ExternalInput")
    if fast_k0:
        xht_d = nc.dram_tensor("xht", [NB, P, NTFB * N], BF,
                               kind="ExternalInput")
    # device layout [b, i, (t, o)]; host permutes back to [b, t, i, o]
    out_d = nc.dram_tensor("out", [NB, N, T * F_OUT], BF, kind="ExternalOutput")

    XW = NTFB * P            # xh8 cols per jb chunk
    MW = NIS * 2 * IS        # m8 cols per jb chunk

    with tile.TileContext(nc) as tc:
        with (
            tc.tile_pool(name="m8", bufs=3) as m8_pool,
            tc.tile_pool(name="xh8", bufs=2) as xh8_pool,
            tc.tile_pool(name="xht", bufs=2) as xht_pool,
            tc.tile_pool(name="rt", bufs=2) as rt_pool,
            tc.tile_pool(name="thp", bufs=1) as thp_pool,
            tc.tile_pool(name="osb", bufs=3) as out_pool,
            tc.tile_pool(name="rtps", bufs=2, space="PSUM") as rtps_pool,
            tc.tile_pool(name="outps", bufs=2, space="PSUM") as outps_pool,
        ):
            import itertools

            # Only SP and Pool issue DMAs: ACT/DVE FIFOs must stay clear for
            # the PSUM->SBUF copies and ReLUs.
            _rr = itertools.count()
            _engs = [nc.sync, nc.gpsimd]

            def dma(dst, src):
                _engs[next(_rr) % 2].dma_start(dst, src)

            thp_sb = thp_pool.tile([P, K * TO], BF, tag="thp")

            m8_tiles = {}
            xh8_tiles = {}
            xht_tiles = {}

            def load_first(b):
                # xh8(b) and m8(b, 0) interleaved per-jb: the first stage-1
                # group's deps (jb=0 chunks) land within ~1.5us.
                xsb = xh8_pool.tile([P, NJ * XW], F8, tag="xh8", name="xh8_sb")
                msb = m8_pool.tile([P, NJ * MW], F8, tag="m8_0", name="m8_0")
                for jb in range(NJ):
                    dma(xsb[:, jb * XW:(jb + 1) * XW],
                        xh8_d.ap()[b][:, jb * XW:(jb + 1) * XW])
                    dma(msb[:, jb * MW:(jb + 1) * MW],
                        m8_d.ap()[b][0][:, jb * MW:(jb + 1) * MW])
                xh8_tiles[b] = xsb
                m8_tiles[(b, 0)] = msb

            def load_m8(b, km, nchunks=4):
                sb = m8_pool.tile([P, NJ * MW], F8, tag=f"m8_{km}",
                                  name=f"m8_{km}")
                cw = NJ * MW // nchunks
                for c in range(nchunks):
                    dma(sb[:, c * cw:(c + 1) * cw],
                        m8_d.ap()[b][km][:, c * cw:(c + 1) * cw])
                m8_tiles[(b, km)] = sb

            def load_xht(b):
                sb = xht_pool.tile([P, NTFB * N], BF, tag="xht", name="xht_sb")
                dma(sb[:], xht_d.ap()[b])
                xht_tiles[b] = sb

            for rep in range(reps):
                if rep == 0:
                    dma(thp_sb[:], thp_d.ap())
                # need-order loads
                for b in range(NB):
                    load_first(b)
                    for km in range(1, KM):
                        load_m8(b, km)
                    if fast_k0:
                        load_xht(b)

                for b in range(NB):
                    xh8_sb = xh8_tiles[b]

                    # stage 1: R_T via fp8 DoubleRow, bf16 out in SBUF
                    rt_sb = rt_pool.tile([P, KM * NTFB * N], BF, tag="rt")
                    cp = itertools.count()
                    for km in range(KM):
                        m8_sb = m8_tiles[(b, km)]
                        for tfb in range(NTFB):
                            rtps = [
                                rtps_pool.tile([P, IS], F32, tag=f"rtps{q}",
                                               name=f"rtps{q}")
                                for q in range(NIS)
                            ]
                            for jb in range(NJ):
                                base_l = jb * XW + tfb * P
                                lhsT = (xh8_sb[:, base_l:base_l + P]
                                        .unsqueeze(1).broadcast_to([P, 2, P]))
                                for q in range(NIS):
                                    base_r = jb * MW + q * 2 * IS
                                    rhs = m8_sb[:, base_r:base_r + 2 * IS].rearrange(
                                        "p (c n) -> p c n", c=2)
                                    nc.tensor.matmul(
                                        rtps[q][:], lhsT, rhs,
                                        start=(jb == 0), stop=(jb == NJ - 1),
                                        perf_mode=DR,
                                    )
                            for q in range(NIS):
                                dst = rt_sb[:, (km * NTFB + tfb) * N + q * IS:
                                            (km * NTFB + tfb) * N + (q + 1) * IS]
                                if next(cp) % 2 == 0:
                                    nc.scalar.copy(dst, rtps[q][:])
                                else:
                                    nc.vector.tensor_copy(dst, rtps[q][:])

                    # stage 2: out[i, (t,o)] accumulated over k per tf block;
                    # two ic's share one osb tile -> one paired store DMA.
                    if fast_k0:
                        xht_sb = xht_tiles[b]
                    for icp in range(NJ // 2):
                        osb = out_pool.tile([P, 2 * T * F_OUT], BF, tag="osb")
                        for half in range(2):
                            ic = icp * 2 + half
                            ops_a = outps_pool.tile([P, 2 * TO], F32,
                                                    tag="outpsA", name="ops_a")
                            ops_b = outps_pool.tile([P, TO], F32,
                                                    tag="outpsB", name="ops_b")
                            for tfb in range(NTFB):
                                for k in range(K):
                                    if fast_k0 and k == 0:
                                        lhs2 = xht_sb[:, tfb * N + ic * P:
                                                      tfb * N + ic * P + P]
                                    else:
                                        km = k - 1 if fast_k0 else k
                                        base = (km * NTFB + tfb) * N + ic * P
                                        lhs2 = rt_sb[:, base:base + P]
                                    dst = (ops_a[:, tfb * TO:(tfb + 1) * TO]
                                           if tfb < 2 else ops_b[:])
                                    nc.tensor.matmul(
                                        dst, lhs2,
                                        thp_sb[:, k * TO:(k + 1) * TO],
                                        start=(k == 0), stop=(k == K - 1),
                                    )
                            off = half * T * F_OUT
                            if ic % 2 == 0:
                                nc.scalar.activation(
                                    osb[:, off:off + 2 * TO], ops_a[:],
                                    mybir.ActivationFunctionType.Relu)
                                nc.vector.tensor_relu(
                                    osb[:, off + 2 * TO:off + 3 * TO], ops_b[:])
                            else:
                                nc.vector.tensor_relu(
                                    osb[:, off:off + 2 * TO], ops_a[:])
                                nc.scalar.activation(
                                    osb[:, off + 2 * TO:off + 3 * TO], ops_b[:],
                                    mybir.ActivationFunctionType.Relu)
                        dma(out_d.ap()[b][icp * 2 * P:(icp + 1) * 2 * P, :]
                            .rearrange("(two p) c -> p two c", two=2),
                            osb[:].rearrange("p (two c) -> p two c", two=2))

    nc.compile()
    return nc


def _pow2_scale(maxabs):
    """Largest power of 2 s with maxabs * s <= 224 (fp8e4m3 headroom)."""
    if maxabs <= 0:
        return 1.0
    return float(2.0 ** np.floor(np.log2(224.0 / maxabs)))


def kernel(x, spatial_attention, cheb, Theta):
    from ml_dtypes import bfloat16, float8_e4m3
    from concourse.bass_utils import run_bass_kernel_spmd

    x = np.asarray(x, dtype=np.float32)
    att = np.asarray(spatial_attention, dtype=np.float32)
    cheb = np.asarray(cheb, dtype=np.float32)
    Theta = np.asarray(Theta, dtype=np.float32)

    fast_k0 = bool(np.abs(cheb[0] - np.eye(N, dtype=np.float32)).max() <= 1e-6)
    kms = list(range(1, K)) if fast_k0 else list(range(K))
    KM = len(kms)

    key = "fast" if fast_k0 else "general"
    if key not in _cache:
        _cache[key] = _build(fast_k0=fast_k0)
    nc = _cache[key]

    # ---- xh8: fp8(x), [b, p, jb*NTFB*P + tfb*P + m] with j = jb*P+p ----
    xh = np.ascontiguousarray(x.transpose(0, 2, 1, 3).reshape(B, N, TF))
    xs = min(_pow2_scale(np.abs(xh).max()) / 32.0, 1.0)
    v = (xh * xs).astype(float8_e4m3)
    xh8 = np.ascontiguousarray(
        v.reshape(B, NJ, P, NTFB, P).transpose(0, 2, 1, 3, 4)
        .reshape(B, P, NJ * NTFB * P))

    # ---- m8: (cheb_k * att_b)^T scaled, fp8 (hi, lo) pair ----
    sk = {}
    m8 = np.empty((B, KM, P, NJ * NIS * 2 * IS), dtype=float8_e4m3)
    for idx, k in enumerate(kms):
        mt = (cheb[k][None] * att).transpose(0, 2, 1)  # [b, j, i]
        s = _pow2_scale(np.abs(mt).max())
        sk[k] = s
        mts = mt * s
        hi = mts.astype(float8_e4m3)
        lo = (mts - hi.astype(np.float32)).astype(float8_e4m3)
        # layout: [b, p, jb, q, c, n] with j = jb*P+p, i = q*IS+n
        pr = np.stack(
            [hi.reshape(B, NJ, P, NIS, IS), lo.reshape(B, NJ, P, NIS, IS)],
            axis=4)  # [b, jb, p, q, c, n]
        m8[:, idx] = pr.transpose(0, 2, 1, 3, 4, 5).reshape(B, P, -1)

    # ---- thetap: block-diag padded Theta with 1/s_k folded in ----
    thp = np.zeros((P, K * TO), dtype=np.float32)
    for kk in range(K):
        if fast_k0 and kk == 0:
            sc = 1.0
        else:
            sc = 1.0 / (sk[kk] * xs)
        for tr in range(TBLK):
            thp[tr * F_IN:(tr + 1) * F_IN,
                kk * TO + tr * F_OUT:kk * TO + (tr + 1) * F_OUT] = Theta[kk] * sc
    thp = thp.astype(bfloat16)

    in_maps = [
        {
            "m8": m8[c * NB:(c + 1) * NB],
            "xh8": xh8[c * NB:(c + 1) * NB],
            "thp": thp,
        }
        for c in range(M_CORES)
    ]
    if fast_k0:
        # xht[b, tf, i] = x[b,t,i,f] * att[b,i,i], packed [b, p, tfb*N + i]
        attd = np.einsum("bii->bi", att)
        w = (x.transpose(0, 1, 3, 2).reshape(B, TF, N) * attd[:, None, :])
        xht = np.ascontiguousarray(
            w.reshape(B, NTFB, P, N).transpose(0, 2, 1, 3).reshape(B, P, NTFB * N)
        ).astype(bfloat16)
        for c in range(M_CORES):
            in_maps[c]["xht"] = xht[c * NB:(c + 1) * NB]

    try:
        res = run_bass_kernel_spmd(nc, in_maps, list(range(M_CORES)))
    except Exception:
        # transient NRT device hiccups recover on redispatch
        res = run_bass_kernel_spmd(nc, in_maps, list(range(M_CORES)))
    out = np.concatenate([res.results[c]["out"] for c in range(M_CORES)], axis=0)
    # device layout [b, i, (t, o)] -> [b, t, i, o], bf16 -> f32
    out = out.astype(np.float32).reshape(B, N, T, F_OUT).transpose(0, 2, 1, 3)
    return np.ascontiguousarray(out)


# revision 30
# speedup vs baseline: 1.2339x; 1.1451x over previous
"""ChebConv-with-spatial-attention Trainium2 kernel (8 NeuronCores, SPMD data-parallel).

Math (per batch b):
    M_k = cheb[k] * att[b]              (elementwise, [N,N])
    R_k = M_k @ xmat[b]                 (xmat[b][j, t*F+f] = x[b,t,j,f], [N, T*F])
    out[b,t,i,o] = relu( sum_k sum_f R_k[i, t*F+f] * Theta[k,f,o] )

Device mapping (per core, 2 batches), fp8 DoubleRow scheme:
    stage 1: host ships Mt = (cheb_k*att_b)^T pre-scaled, quantized as an
             fp8e4m3 (hi, lo) pair with lo = fp8(M - hi), so hi+lo carries
             ~bf16 accuracy.  PE DoubleRow matmuls contract both slots at
             0.5 cycles/column:
                 R_T[tf, i] += xh8[j, tf].T (bcast slots) @ Mpair[j, c, i]
             with xh8 = fp8(x) broadcast across the two slots (stride-0 AP).
             This halves stage-1 PE time vs bf16 while keeping max-rel error
             ~1.4e-2 (dominated by the single fp8 rounding of x).
    stage 2: bf16 PE matmuls out[i, (t,o)] += R_T[tf_blk, i].T @ thetap[k]
             with thetap a block-diagonal padded Theta ([128, 4*64] per k,
             scale 1/s_k folded in), accumulated over k in PSUM; fused ReLU
             on copy-out; bf16 output stores (host casts back to f32).

Host pre-processing: all inputs packed as SBUF images [128, cols] so every
DMA moves >=512B contiguous rows at the full modeled bandwidth; loads are
issued in need-order with (xh8, m8) chunk interleaving so the PE starts
within ~2us.  T_0 of any Chebyshev basis is the identity, so its term
reduces to a diagonal (attention-diag) scaling folded into `xht` (bf16).
"""

import numpy as np

B, T, N, F_IN, F_OUT, K = 16, 12, 1024, 32, 64, 3
M_CORES = 8
NB = B // M_CORES          # batches per core
P = 128                    # SBUF partitions
NJ = N // P                # 8 contraction chunks
TF = T * F_IN              # 384
NTFB = TF // P             # 3 tf blocks
TBLK = P // F_IN           # 4 t's per tf block
IS = 512                   # stage-1 moving width
NIS = N // IS              # 2 i strips
TO = TBLK * F_OUT          # 256 = stage-2 rhs width

_cache = {}


def _build(fast_k0=True, reps=1):
    import concourse.bacc as bacc
    import concourse.mybir as mybir
    import concourse.tile as tile

    F8 = mybir.dt.float8e4
    BF = mybir.dt.bfloat16
    F32 = mybir.dt.float32
    DR = mybir.MatmulPerfMode.DoubleRow
    KM = K - 1 if fast_k0 else K   # k's that need stage-1 matmuls

    nc = bacc.Bacc("TRN2", target_bir_lowering=False, debug=False)
    m8_d = nc.dram_tensor("m8", [NB, KM, P, NJ * NIS * 2 * IS], F8,
                          kind="ExternalInput")
    xh8_d = nc.dram_tensor("xh8", [NB, P, NJ * NTFB * P], F8,
                           kind="ExternalInput")
    thp_d = nc.dram_tensor("thp", [P, K * TO], BF, kind="ExternalInput")
    if fast_k0:
        xht_d = nc.dram_tensor("xht", [NB, P, NTFB * N], BF,
                               kind="ExternalInput")
    # device layout [b, i, (t, o)]; host permutes back to [b, t, i, o]
    out_d = nc.dram_tensor("out", [NB, N, T * F_OUT], BF, kind="ExternalOutput")

    XW = NTFB * P            # xh8 cols per jb chunk
    MW = NIS * 2 * IS        # m8 cols per jb chunk

    with tile.TileContext(nc) as tc:
        with (
            tc.tile_pool(name="m8", bufs=3) as m8_pool,
            tc.tile_pool(name="xh8", bufs=2) as xh8_pool,
            tc.tile_pool(name="xht", bufs=2) as xht_pool,
            tc.tile_pool(name="rt", bufs=2) as rt_pool,
            tc.tile_pool(name="thp", bufs=1) as thp_pool,
            tc.tile_pool(name="osb", bufs=3) as out_pool,
            tc.tile_pool(name="wrm", bufs=1) as wrm_pool,
            tc.tile_pool(name="rtps", bufs=1, space="PSUM") as rtps_pool,
            tc.tile_pool(name="outps", bufs=2, space="PSUM") as outps_pool,
        ):
            import itertools

            # Loads round-robin on SP+Pool (parallel DGE generation).  Stores
            # ride the same queues: they sit behind all loads in program
            # order, so they never delay the input stream, and ACT/DVE FIFOs
            # stay clear for copies and ReLUs.
            _rr = itertools.count()
            _engs = [nc.sync, nc.gpsimd]

            def dma(dst, src):
                _engs[next(_rr) % 2].dma_start(dst, src)

            dma_store = dma

            thp_sb = thp_pool.tile([P, K * TO], BF, tag="thp")

            m8_tiles = {}
            xh8_tiles = {}
            xht_tiles = {}

            def load_xh8(b, nchunks=1):
                sb = xh8_pool.tile([P, NJ * XW], F8, tag="xh8", name="xh8_sb")
                cw = NJ * XW // nchunks
                for c in range(nchunks):
                    dma(sb[:, c * cw:(c + 1) * cw],
                        xh8_d.ap()[b][:, c * cw:(c + 1) * cw])
                xh8_tiles[b] = sb

            def load_m8(b, km, nchunks=4):
                sb = m8_pool.tile([P, NJ * MW], F8, tag=f"m8_{km}",
                                  name=f"m8_{km}")
                cw = NJ * MW // nchunks
                for c in range(nchunks):
                    dma(sb[:, c * cw:(c + 1) * cw],
                        m8_d.ap()[b][km][:, c * cw:(c + 1) * cw])
                m8_tiles[(b, km)] = sb

            def load_xht(b):
                sb = xht_pool.tile([P, NTFB * N], BF, tag="xht", name="xht_sb")
                dma(sb[:], xht_d.ap()[b])
                xht_tiles[b] = sb

            _s1ps = itertools.count()
            _cp = itertools.count()
            NCH = 4                       # m8 DMA chunks per (b, km)
            JPC = NJ // NCH               # jb's per chunk

            def s1_begin():
                # 4 rotating PSUM banks so consecutive q-passes never wait
                # on the previous pass's copy-out.
                return [
                    rtps_pool.tile([P, IS], F32,
                                   tag=f"rtps{next(_s1ps) % 4}", name="rtps")
                    for _ in range(NTFB)
                ]

            def s1_chunk(b, km, q, c, rtps):
                xh8_sb = xh8_tiles[b]
                m8_sb = m8_tiles[(b, km)]
                for jb in range(c * JPC, (c + 1) * JPC):
                    base_r = jb * MW + q * 2 * IS
                    rhs = m8_sb[:, base_r:base_r + 2 * IS].rearrange(
                        "p (c n) -> p c n", c=2)
                    for tfb in range(NTFB):
                        base_l = jb * XW + tfb * P
                        lhsT = (xh8_sb[:, base_l:base_l + P]
                                .unsqueeze(1).broadcast_to([P, 2, P]))
                        nc.tensor.matmul(
                            rtps[tfb][:], lhsT, rhs,
                            start=(jb == 0), stop=(jb == NJ - 1),
                            perf_mode=DR,
                        )

            def s1_end(b, km, q, rtps):
                rt_sb = rt_tiles[b]
                for tfb in range(NTFB):
                    dst = rt_sb[:, (km * NTFB + tfb) * N + q * IS:
                                (km * NTFB + tfb) * N + (q + 1) * IS]
                    if next(_cp) % 2 == 0:
                        nc.scalar.copy(dst, rtps[tfb][:])
                    else:
                        nc.vector.tensor_copy(dst, rtps[tfb][:])

            def s1_pass(b, km, q):
                rtps = s1_begin()
                for c in range(NCH):
                    s1_chunk(b, km, q, c, rtps)
                s1_end(b, km, q, rtps)

            def s2_units(b, korder):
                # generator of per-ic emission units: out[i, (t,o)]
                # accumulated over k per tf block, in korder (the k whose
                # operand loads last goes last).  Two ic's share one osb
                # tile -> one paired store DMA on ACT's queue.
                rt_sb = rt_tiles[b]
                xht_sb = xht_tiles[b] if fast_k0 else None
                osb = None
                for ic in range(NJ):
                    if ic % 2 == 0:
                        osb = out_pool.tile([P, 2 * T * F_OUT], BF, tag="osb",
                                            name="osb")
                    ops_a = outps_pool.tile([P, 2 * TO], F32,
                                            tag="outpsA", name="ops_a")
                    ops_b = outps_pool.tile([P, TO], F32,
                                            tag="outpsB", name="ops_b")
                    for tfb in range(NTFB):
                        for ki, k in enumerate(korder):
                            if fast_k0 and k == 0:
                                lhs2 = xht_sb[:, tfb * N + ic * P:
                                              tfb * N + ic * P + P]
                            else:
                                km = k - 1 if fast_k0 else k
                                base = (km * NTFB + tfb) * N + ic * P
                                lhs2 = rt_sb[:, base:base + P]
                            dst = (ops_a[:, tfb * TO:(tfb + 1) * TO]
                                   if tfb < 2 else ops_b[:])
                            nc.tensor.matmul(
                                dst, lhs2, thp_sb[:, k * TO:(k + 1) * TO],
                                start=(ki == 0), stop=(ki == K - 1),
                            )
                    off = (ic % 2) * T * F_OUT
                    if ic % 2 == 0:
                        nc.scalar.activation(
                            osb[:, off:off + 2 * TO], ops_a[:],
                            mybir.ActivationFunctionType.Relu)
                        nc.vector.tensor_relu(
                            osb[:, off + 2 * TO:off + 3 * TO], ops_b[:])
                    else:
                        nc.vector.tensor_relu(
                            osb[:, off:off + 2 * TO], ops_a[:])
                        nc.scalar.activation(
                            osb[:, off + 2 * TO:off + 3 * TO], ops_b[:],
                            mybir.ActivationFunctionType.Relu)
                        icp = ic // 2
                        dma_store(
                            out_d.ap()[b][icp * 2 * P:(icp + 1) * 2 * P, :]
                            .rearrange("(two p) c -> p two c", two=2),
                            osb[:].rearrange("p (two c) -> p two c", two=2))
                    yield

            for rep in range(reps):
                rt_tiles = {}
                # need-order loads.  b1 is asymmetric: km1 before km0 and
                # xht(b1) dead last, so stage-2(b1)'s k2/k1 matmuls overlap
                # the xht transfer and only the cheap k0 matmuls + relu +
                # store chain sits behind the final input bytes.
                if rep == 0:
                    dma(thp_sb[:], thp_d.ap())
                load_xh8(0, nchunks=1)
                for km in range(KM):
                    load_m8(0, km)
                if fast_k0:
                    load_xht(0)
                load_xh8(1)
                for km in reversed(range(KM)):
                    load_m8(1, km)
                if fast_k0:
                    load_xht(1)

                for b in range(NB):
                    rt_tiles[b] = rt_pool.tile([P, KM * NTFB * N], BF, tag="rt",
                                               name="rt_sb")

                if fast_k0:
                    # Warm-up: dummy matmuls on a memset tile keep the PE
                    # busy (and its p-state ramp warm) from t~0 while the
                    # first real chunks stream in.
                    if rep == 0:
                        dsrc = wrm_pool.tile([P, P], BF, tag="wrm",
                                             name="dsrc")
                        nc.vector.memset(dsrc[:], 0)
                        dps = outps_pool.tile([P, TO], F32, tag="outpsB",
                                              name="dps")
                        for i in range(40):
                            nc.tensor.matmul(
                                dps[:, :64], dsrc[:], dsrc[:, :64],
                                start=True, stop=True,
                            )

                    # b0: plain.  b1: km1 first (matches its load order) with
                    # s2(b0) ic-groups interleaved ahead of the feed-gated q0
                    # chunk groups, keeping the PE stream continuous.
                    for km in range(KM):
                        for q in range(NIS):
                            s1_pass(0, km, q)
                    s2b0 = s2_units(0, [1, 2, 0])

                    def pump(n):
                        for _ in range(n):
                            next(s2b0, None)

                    for km in reversed(range(KM)):
                        for q in range(NIS):
                            rtps = s1_begin()
                            for c in range(NCH):
                                if q == 0:
                                    pump(1)
                                s1_chunk(1, km, q, c, rtps)
                            s1_end(1, km, q, rtps)
                            if q == 1:
                                pump(1)
                    pump(NJ)
                    for _ in s2_units(1, [2, 1, 0]):
                        pass
                else:
                    for b in range(NB):
                        for km in range(KM):
                            for q in range(NIS):
                                s1_pass(b, km, q)
                        for _ in s2_units(b, list(range(K))):
                            pass

    nc.compile()
    return nc


def _pow2_scale(maxabs):
    """Largest power of 2 s with maxabs * s <= 224 (fp8e4m3 headroom)."""
    if maxabs <= 0:
        return 1.0
    return float(2.0 ** np.floor(np.log2(224.0 / maxabs)))


def kernel(x, spatial_attention, cheb, Theta):
    from ml_dtypes import bfloat16, float8_e4m3
    from concourse.bass_utils import run_bass_kernel_spmd

    x = np.asarray(x, dtype=np.float32)
    att = np.asarray(spatial_attention, dtype=np.float32)
    cheb = np.asarray(cheb, dtype=np.float32)
    Theta = np.asarray(Theta, dtype=np.float32)

    fast_k0 = bool(np.abs(cheb[0] - np.eye(N, dtype=np.float32)).max() <= 1e-6)
    kms = list(range(1, K)) if fast_k0 else list(range(K))
    KM = len(kms)

    key = "fast" if fast_k0 else "general"
    if key not in _cache:
        _cache[key] = _build(fast_k0=fast_k0)
    nc = _cache[key]

    # ---- xh8: fp8(x), [b, p, jb*NTFB*P + tfb*P + m] with j = jb*P+p ----
    xh = np.ascontiguousarray(x.transpose(0, 2, 1, 3).reshape(B, N, TF))
    xs = min(_pow2_scale(np.abs(xh).max()) / 32.0, 1.0)
    v = (xh * xs).astype(float8_e4m3)
    xh8 = np.ascontiguousarray(
        v.reshape(B, NJ, P, NTFB, P).transpose(0, 2, 1, 3, 4)
        .reshape(B, P, NJ * NTFB * P))

    # ---- m8: (cheb_k * att_b)^T scaled, fp8 (hi, lo) pair ----
    sk = {}
    m8 = np.empty((B, KM, P, NJ * NIS * 2 * IS), dtype=float8_e4m3)
    for idx, k in enumerate(kms):
        mt = (cheb[k][None] * att).transpose(0, 2, 1)  # [b, j, i]
        s = _pow2_scale(np.abs(mt).max())
        sk[k] = s
        mts = mt * s
        hi = mts.astype(float8_e4m3)
        lo = (mts - hi.astype(np.float32)).astype(float8_e4m3)
        # layout: [b, p, jb, q, c, n] with j = jb*P+p, i = q*IS+n
        pr = np.stack(
            [hi.reshape(B, NJ, P, NIS, IS), lo.reshape(B, NJ, P, NIS, IS)],
            axis=4)  # [b, jb, p, q, c, n]
        m8[:, idx] = pr.transpose(0, 2, 1, 3, 4, 5).reshape(B, P, -1)

    # ---- thetap: block-diag padded Theta with 1/s_k folded in ----
    thp = np.zeros((P, K * TO), dtype=np.float32)
    for kk in range(K):
        if fast_k0 and kk == 0:
            sc = 1.0
        else:
            sc = 1.0 / (sk[kk] * xs)
        for tr in range(TBLK):
            thp[tr * F_IN:(tr + 1) * F_IN,
                kk * TO + tr * F_OUT:kk * TO + (tr + 1) * F_OUT] = Theta[kk] * sc
    thp = thp.astype(bfloat16)

    in_maps = [
        {
            "m8": m8[c * NB:(c + 1) * NB],
            "xh8": xh8[c * NB:(c + 1) * NB],
            "thp": thp,
        }
        for c in range(M_CORES)
    ]
    if fast_k0:
        # xht[b, tf, i] = x[b,t,i,f] * att[b,i,i], packed [b, p, tfb*N + i]
        attd = np.einsum("bii->bi", att)
        w = (x.transpose(0, 1, 3, 2).reshape(B, TF, N) * attd[:, None, :])
        xht = np.ascontiguousarray(
            w.reshape(B, NTFB, P, N).transpose(0, 2, 1, 3).reshape(B, P, NTFB * N)
        ).astype(bfloat16)
        for c in range(M_CORES):
            in_maps[c]["xht"] = xht[c * NB:(c + 1) * NB]

    try:
        res = run_bass_kernel_spmd(nc, in_maps, list(range(M_CORES)))
    except Exception:
        # transient NRT device hiccups recover on redispatch
        res = run_bass_kernel_spmd(nc, in_maps, list(range(M_CORES)))
    out = np.concatenate([res.results[c]["out"] for c in range(M_CORES)], axis=0)
    # device layout [b, i, (t, o)] -> [b, t, i, o], bf16 -> f32
    out = out.astype(np.float32).reshape(B, N, T, F_OUT).transpose(0, 2, 1, 3)
    return np.ascontiguousarray(out)
